# revision 1
# baseline (speedup 1.0000x reference)
"""CrossAttention kernel for 8 TRN2 NeuronCores (data-parallel over batch).

Per batch element b (one core each):
  q = Wq @ x_flat                  # [512, 4096]
  kT = (SCALE * Wk) @ ctx.T        # [512, 256]
  v = ctx @ Wv.T                   # [256, 512]
  per head h (8 heads x 64 dim), j = context pos in partitions:
    simT_h = kT_h.T @ q_h          # [256, i]
    E = exp(simT_h)                # no max-subtract: |sim| < ~2
    out_h = (v_h.T @ E) / (1.T E)  # attn@v + ones-matmul denominator
  final = Wout @ outcat + (x + bout)   # bias folded into residual on host

All matmuls bf16 (fp32 accumulation in PSUM).  i chunked by CH=512.
attn@v / denominator use col tile_position pairs (even head -> psum rows
0-63, odd -> 64-127), denominator replicated across 64 partitions by an
all-ones [128,64] stationary, so normalization is full-width on DVE.
"""

import numpy as np
import ml_dtypes

import concourse.bass as bass
import concourse.mybir as mybir
import concourse.tile as tile
from concourse import bacc
from concourse.bass_utils import run_bass_kernel_spmd

HEADS = 8
DIM_HEAD = 64
SCALE = DIM_HEAD ** -0.5
DIM = 512          # channels of x
CTX_DIM = 768
N_CTX = 256        # context positions
HW = 4096          # 64*64 pixels
CH = 512           # i-chunk size
NCHUNK = HW // CH  # 8
B = 8              # batch == number of cores

F32 = mybir.dt.float32
BF16 = mybir.dt.bfloat16


def build_bass(loop_n=1):
    nc = bacc.Bacc(
        "TRN2",
        target_bir_lowering=False,
        debug=False,
        num_devices=B,
    )

    # DRAM parameters (per-core shard shapes)
    xres_d = nc.declare_dram_parameter("xres", [DIM, HW], F32, isOutput=False)
    xb_d = nc.declare_dram_parameter("xb", [DIM, HW], BF16, isOutput=False)
    ctxT_d = nc.declare_dram_parameter("ctxT", [CTX_DIM, N_CTX], BF16, isOutput=False)
    wqT_d = nc.declare_dram_parameter("wqT", [DIM, DIM], BF16, isOutput=False)
    wkT_d = nc.declare_dram_parameter("wkT", [CTX_DIM, DIM], BF16, isOutput=False)
    wvT_d = nc.declare_dram_parameter("wvT", [CTX_DIM, DIM], BF16, isOutput=False)
    woutT_d = nc.declare_dram_parameter("woutT", [DIM, DIM], BF16, isOutput=False)
    out_d = nc.declare_dram_parameter("out", [DIM, HW], F32, isOutput=True)

    # DRAM views tiled to 128 partitions
    xres_t = xres_d[:].rearrange("(t p) i -> p t i", p=128)   # [128, 4, 4096]
    xb_t = xb_d[:].rearrange("(t p) i -> p t i", p=128)       # [128, 4, 4096]
    ctxT_t = ctxT_d[:].rearrange("(t p) n -> p t n", p=128)   # [128, 6, 256]
    wqT_t = wqT_d[:].rearrange("(t p) e -> p t e", p=128)     # [128, 4, 512]
    wkT_t = wkT_d[:].rearrange("(t p) e -> p t e", p=128)     # [128, 6, 512]
    wvT_t = wvT_d[:].rearrange("(t p) e -> p t e", p=128)     # [128, 6, 512]
    woutT_t = woutT_d[:].rearrange("(t p) c -> p t c", p=128)  # [128, 4, 512]
    out_t = out_d[:].rearrange("(t p) i -> p t i", p=128)     # [128, 4, 4096]

    with tile.TileContext(nc) as tc:
        with (
            tc.tile_pool(name="wts", bufs=1) as wts,
            tc.tile_pool(name="kv", bufs=1) as kvp,
            tc.tile_pool(name="xp", bufs=3) as xp,
            tc.tile_pool(name="qp", bufs=2) as qp,
            tc.tile_pool(name="ep", bufs=3) as ep,
            tc.tile_pool(name="rp", bufs=3) as rp,
            tc.tile_pool(name="ocp", bufs=2) as ocp,
            tc.tile_pool(name="outp", bufs=2) as outp,
            tc.tile_pool(name="ps", bufs=4, space="PSUM") as ps,
            tc.tile_pool(name="ps2", bufs=2, space="PSUM") as ps2,
        ):
            # ---- load weights / context ----
            wq_sb = wts.tile([128, 4, DIM], BF16)
            nc.gpsimd.dma_start(out=wq_sb, in_=wqT_t)
            wk_sb = wts.tile([128, 6, DIM], BF16)
            nc.gpsimd.dma_start(out=wk_sb, in_=wkT_t)
            wv_sb = wts.tile([128, 6, DIM], BF16)
            nc.gpsimd.dma_start(out=wv_sb, in_=wvT_t)
            wo_sb = wts.tile([128, 4, DIM], BF16)
            nc.gpsimd.dma_start(out=wo_sb, in_=woutT_t)
            ctx_sb = wts.tile([128, 6, N_CTX], BF16)
            nc.gpsimd.dma_start(out=ctx_sb, in_=ctxT_t)
            ones_sb = wts.tile([128, DIM_HEAD], BF16)
            nc.vector.memset(ones_sb, 1.0)

            # loop_n > 1 repeats the whole compute for slope-based timing
            for _it in range(loop_n):
                # ---- kT = WkT.T @ ctxT : [512, 256] as [128, 4, 256] ----
                kT_sb = kvp.tile([128, 4, N_CTX], BF16, tag="kT")
                for m in range(4):
                    pt = ps.tile([128, CH], F32, tag="ps")
                    for k in range(6):
                        nc.tensor.matmul(
                            pt[:, :N_CTX],
                            wk_sb[:, k, bass.ts(m, 128)],
                            ctx_sb[:, k, :],
                            start=(k == 0),
                            stop=(k == 5),
                        )
                    nc.scalar.copy(out=kT_sb[:, m, :], in_=pt[:, :N_CTX])

                # ---- v = ctxT.T @ WvT : [256, 512] as [128, 2, 512] ----
                v_sb = kvp.tile([128, 2, DIM], BF16, tag="v")
                for m in range(2):
                    pt = ps.tile([128, CH], F32, tag="ps")
                    for k in range(6):
                        nc.tensor.matmul(
                            pt,
                            ctx_sb[:, k, bass.ts(m, 128)],
                            wv_sb[:, k, :],
                            start=(k == 0),
                            stop=(k == 5),
                        )
                    nc.scalar.copy(out=v_sb[:, m, :], in_=pt)

                # ---- main loop over pixel chunks ----
                for c in range(NCHUNK):
                    isl = bass.ts(c, CH)

                    xb_sb = xp.tile([128, 4, CH], BF16, tag="xb")
                    nc.gpsimd.dma_start(out=xb_sb, in_=xb_t[:, :, isl])
                    xr_sb = xp.tile([128, 4, CH], F32, tag="xr")
                    nc.gpsimd.dma_start(out=xr_sb, in_=xres_t[:, :, isl])

                    # q = WqT.T @ xb  -> [128, 4, CH] (e tiles), bf16
                    q_sb = qp.tile([128, 4, CH], BF16)
                    for m in range(4):
                        pt = ps.tile([128, CH], F32, tag="ps")
                        for k in range(4):
                            nc.tensor.matmul(
                                pt,
                                wq_sb[:, k, bass.ts(m, 128)],
                                xb_sb[:, k, :],
                                start=(k == 0),
                                stop=(k == 3),
                            )
                        nc.vector.tensor_copy(out=q_sb[:, m, :], in_=pt)

                    # per head-pair attention
                    oc_sb = ocp.tile([128, 4, CH], BF16)
                    for p in range(4):  # head pair p -> heads 2p, 2p+1
                        # simT for both heads: [128(j), 2, CH] psum (2 banks),
                        # interleaved even/odd for row-group concurrency
                        pts = [ps2.tile([128, 2, CH], F32, tag="sim",
                                        name=f"psim{p}_{hh2}")
                               for hh2 in range(2)]
                        for j in range(2):
                            for hh in range(2):
                                h0 = hh * 64
                                nc.tensor.matmul(
                                    pts[hh][:, j, :],
                                    kT_sb[h0:h0 + 64, p, bass.ts(j, 128)],
                                    q_sb[h0:h0 + 64, p, :],
                                    start=True,
                                    stop=True,
                                )
                        # exp: one ACT op per head over both j tiles
                        e_tiles = []
                        for hh in range(2):
                            e_sb = ep.tile([128, 2, CH], BF16, tag="e")
                            nc.scalar.activation(
                                out=e_sb,
                                in_=pts[hh],
                                func=mybir.ActivationFunctionType.Exp,
                            )
                            e_tiles.append(e_sb)

                        # attn@v + denominator, col-group pairs
                        pav = ps.tile([128, CH], F32, tag="ps")
                        pS = ps.tile([128, CH], F32, tag="ps")
                        for kj in range(2):
                            for hh in range(2):
                                h = 2 * p + hh
                                h0 = hh * 64
                                nc.tensor.matmul(
                                    pav[h0:h0 + 64, :],
                                    v_sb[:, kj, bass.ds(h * 64, 64)],
                                    e_tiles[hh][:, kj, :],
                                    start=(kj == 0),
                                    stop=(kj == 1),
                                    skip_group_check=True,
                                )
                        for kj in range(2):
                            for hh in range(2):
                                h0 = hh * 64
                                nc.tensor.matmul(
                                    pS[h0:h0 + 64, :],
                                    ones_sb,
                                    e_tiles[hh][:, kj, :],
                                    start=(kj == 0),
                                    stop=(kj == 1),
                                    skip_group_check=True,
                                )
                        # normalize: outcat = pav / pS  (full 128-width)
                        r_sb = rp.tile([128, CH], F32, tag="r")
                        nc.vector.reciprocal_approx_fast(out=r_sb, in_=pS)
                        nc.vector.tensor_mul(out=oc_sb[:, p, :], in0=pav, in1=r_sb)

                    # out projection + (residual + bias)
                    o_sb = outp.tile([128, 4, CH], F32)
                    for m in range(4):
                        pt = ps.tile([128, CH], F32, tag="ps")
                        for k in range(4):
                            nc.tensor.matmul(
                                pt,
                                wo_sb[:, k, bass.ts(m, 128)],
                                oc_sb[:, k, :],
                                start=(k == 0),
                                stop=(k == 3),
                            )
                        nc.vector.tensor_add(
                            out=o_sb[:, m, :],
                            in0=pt,
                            in1=xr_sb[:, m, :],
                        )
                    nc.gpsimd.dma_start(out=out_t[:, :, isl], in_=o_sb)

    nc.compile()
    return nc


_NC_CACHE = None


def _get_nc():
    global _NC_CACHE
    if _NC_CACHE is None:
        _NC_CACHE = build_bass()
    return _NC_CACHE


def make_in_maps(x, context, Wq, Wkv, Wout, bout):
    """Host-side prep: shard over batch, pre-transpose weights, cast bf16."""
    f = np.float32
    bf = ml_dtypes.bfloat16
    wqT = np.ascontiguousarray(Wq.T).astype(bf)
    wkT = np.ascontiguousarray(Wkv[:512].T * np.float32(SCALE)).astype(bf)
    wvT = np.ascontiguousarray(Wkv[512:].T).astype(bf)
    woutT = np.ascontiguousarray(Wout.T).astype(bf)
    bout = np.asarray(bout, dtype=f)
    in_maps = []
    for b in range(B):
        xf = np.ascontiguousarray(x[b].reshape(DIM, HW), dtype=f)
        in_maps.append({
            "xres": xf + bout[:, None],
            "xb": xf.astype(bf),
            "ctxT": np.ascontiguousarray(context[b].T).astype(bf),
            "wqT": wqT,
            "wkT": wkT,
            "wvT": wvT,
            "woutT": woutT,
        })
    return in_maps


def kernel(x, context, Wq, Wkv, Wout, bout):
    x = np.asarray(x)
    context = np.asarray(context)
    nc = _get_nc()
    in_maps = make_in_maps(x, context, np.asarray(Wq), np.asarray(Wkv),
                           np.asarray(Wout), np.asarray(bout))
    res = run_bass_kernel_spmd(nc, in_maps, core_ids=list(range(B)))
    out = np.stack([res.results[b]["out"] for b in range(B)], axis=0)
    return out.reshape(B, DIM, 64, 64).astype(np.float32)



# revision 9
# speedup vs baseline: 1.2034x; 1.2034x over previous
"""CrossAttention kernel for 8 TRN2 NeuronCores (data-parallel over batch).

fp8(e4m3) + DoubleRow version. Per batch element b (one core each):
  q32 = (32*Wq)_fp8 @ x_fp8            # [512, 4096] psum, DoubleRow K=256
  kT32 = (32*Wk)_fp8 @ ctx_fp8.T       # [512, 256]  psum, DoubleRow
  v = ((ctx_fp8 @ (32*Wv)_fp8.T)/32)   # [256, 512] -> fp8
  per head h:
    simT32 = kT32_h.T @ q32_h          # bf16 matmul (sim is output-rate bound)
    E = exp(simT32 * SCALE/1024) fp8   # descale folded into ACT scale
    av = v_h.T @ E                     # DoubleRow K=256, one E pass
    S  = ones.T @ E                    # DoubleRow, S replicated over 64 rows
    oc_h = (av / S) fp8
  o32 = (32*Wout)_fp8 @ oc + 32*(x+bout)   # bf16 out; host divides by 32

All projections use fp8 DoubleRow (2 fp8 MACs/cell/cycle). Weights are
quantized x32 on host to stay clear of e4m3 subnormals; descales are folded
into existing casts (free) and into the final host-side /32.
"""

import numpy as np
import ml_dtypes

import concourse.bass as bass
import concourse.mybir as mybir
import concourse.tile as tile
from concourse import bacc
from concourse.bass_utils import run_bass_kernel_spmd

HEADS = 8
DIM_HEAD = 64
SCALE = DIM_HEAD ** -0.5
DIM = 512          # channels of x
CTX_DIM = 768
N_CTX = 256        # context positions
HW = 4096          # 64*64 pixels
CH = 512           # i-chunk size
NCHUNK = HW // CH  # 8
B = 8              # batch == number of cores

F32 = mybir.dt.float32
BF16 = mybir.dt.bfloat16
F8 = mybir.dt.float8e4
DR = mybir.MatmulPerfMode.DoubleRow


def build_bass(loop_n=1):
    nc = bacc.Bacc(
        "TRN2",
        target_bir_lowering=False,
        debug=False,
        num_devices=B,
    )

    # DRAM parameters (per-core shard shapes)
    x8_d = nc.declare_dram_parameter("x8", [DIM, HW], F8, isOutput=False)
    xres_d = nc.declare_dram_parameter("xres32", [DIM, HW], BF16, isOutput=False)
    ctxT_d = nc.declare_dram_parameter("ctxT8", [CTX_DIM, N_CTX], F8, isOutput=False)
    wqT_d = nc.declare_dram_parameter("wqT8", [DIM, DIM], F8, isOutput=False)
    wkT_d = nc.declare_dram_parameter("wkT8", [CTX_DIM, DIM], F8, isOutput=False)
    wvT_d = nc.declare_dram_parameter("wvT8", [CTX_DIM, DIM], F8, isOutput=False)
    woutT_d = nc.declare_dram_parameter("woT8", [DIM, DIM], F8, isOutput=False)
    out_d = nc.declare_dram_parameter("out", [DIM, HW], BF16, isOutput=True)

    # DRAM views tiled to 128 partitions
    x8_t = x8_d[:].rearrange("(t p) i -> p t i", p=128)       # [128, 4, 4096]
    xres_t = xres_d[:].rearrange("(t p) i -> p t i", p=128)   # [128, 4, 4096]
    ctxT_t = ctxT_d[:].rearrange("(t p) n -> p t n", p=128)   # [128, 6, 256]
    wqT_t = wqT_d[:].rearrange("(t p) e -> p t e", p=128)     # [128, 4, 512]
    wkT_t = wkT_d[:].rearrange("(t p) e -> p t e", p=128)     # [128, 6, 512]
    wvT_t = wvT_d[:].rearrange("(t p) e -> p t e", p=128)     # [128, 6, 512]
    woutT_t = woutT_d[:].rearrange("(t p) c -> p t c", p=128)  # [128, 4, 512]
    out_t = out_d[:].rearrange("(t p) i -> p t i", p=128)     # [128, 4, 4096]

    with tile.TileContext(nc) as tc:
        with (
            tc.tile_pool(name="wts", bufs=1) as wts,
            tc.tile_pool(name="kv", bufs=1) as kvp,
            tc.tile_pool(name="xp", bufs=3) as xp,
            tc.tile_pool(name="qp", bufs=2) as qp,
            tc.tile_pool(name="ep", bufs=3) as ep,
            tc.tile_pool(name="rp", bufs=3) as rp,
            tc.tile_pool(name="ocp", bufs=2) as ocp,
            tc.tile_pool(name="outp", bufs=2) as outp,
            tc.tile_pool(name="ps", bufs=2, space="PSUM") as ps,
            tc.tile_pool(name="ps2", bufs=3, space="PSUM") as ps2,
        ):
            # ---- load weights / context (kT deps first) ----
            wk_sb = wts.tile([128, 6, DIM], F8)
            nc.gpsimd.dma_start(out=wk_sb, in_=wkT_t)
            ctx_sb = wts.tile([128, 6, N_CTX], F8)
            nc.gpsimd.dma_start(out=ctx_sb, in_=ctxT_t)
            ones_sb = wts.tile([128, DIM_HEAD], BF16)
            nc.vector.memset(ones_sb, 1.0)
            wv_sb = wts.tile([128, 6, DIM], F8)
            nc.gpsimd.dma_start(out=wv_sb, in_=wvT_t)
            wq_sb = wts.tile([128, 4, DIM], F8)
            nc.gpsimd.dma_start(out=wq_sb, in_=wqT_t)
            wo_sb = wts.tile([128, 4, DIM], F8)
            nc.gpsimd.dma_start(out=wo_sb, in_=woutT_t)

            # loop_n > 1 repeats the whole compute for slope-based timing
            for _it in range(loop_n):
                # ---- kT32 = (32WkT).T @ ctxT8 : [512, 256] as [128, 4, 256] bf16
                kT_sb = kvp.tile([128, 4, N_CTX], BF16, tag="kT")
                for m in range(4):
                    pt = ps.tile([128, CH], F32, tag="ps")
                    for u in range(3):
                        nc.tensor.matmul(
                            pt[:, :N_CTX],
                            wk_sb[:, 2 * u:2 * u + 2, bass.ts(m, 128)],
                            ctx_sb[:, 2 * u:2 * u + 2, :],
                            start=(u == 0),
                            stop=(u == 2),
                            perf_mode=DR,
                        )
                    nc.scalar.copy(out=kT_sb[:, m, :], in_=pt[:, :N_CTX])

                # ---- v = (ctxT8.T @ 32WvT)/32 : [256, 512] bf16 as [128, 2, 512]
                v_sb = kvp.tile([128, 2, DIM], BF16, tag="v")
                for m in range(2):
                    pt = ps.tile([128, CH], F32, tag="ps")
                    for u in range(3):
                        nc.tensor.matmul(
                            pt,
                            ctx_sb[:, 2 * u:2 * u + 2, bass.ts(m, 128)],
                            wv_sb[:, 2 * u:2 * u + 2, :],
                            start=(u == 0),
                            stop=(u == 2),
                            perf_mode=DR,
                        )
                    nc.scalar.mul(out=v_sb[:, m, :], in_=pt, mul=1.0 / 32)

                # ---- main loop over pixel chunks ----
                for c in range(NCHUNK):
                    isl = bass.ts(c, CH)

                    x8_sb = xp.tile([128, 4, CH], F8, tag="x8")
                    nc.gpsimd.dma_start(out=x8_sb, in_=x8_t[:, :, isl])
                    xr_sb = xp.tile([128, 4, CH], BF16, tag="xr")
                    nc.gpsimd.dma_start(out=xr_sb, in_=xres_t[:, :, isl])

                    # q32 = (32WqT).T @ x8 -> [128, 4, CH] bf16 (carries x32)
                    q_sb = qp.tile([128, 4, CH], BF16)
                    for m in range(4):
                        pt = ps.tile([128, CH], F32, tag="ps")
                        for u in range(2):
                            nc.tensor.matmul(
                                pt,
                                wq_sb[:, 2 * u:2 * u + 2, bass.ts(m, 128)],
                                x8_sb[:, 2 * u:2 * u + 2, :],
                                start=(u == 0),
                                stop=(u == 1),
                                perf_mode=DR,
                            )
                        nc.vector.tensor_copy(out=q_sb[:, m, :], in_=pt)

                    # per head-pair attention
                    oc_sb = ocp.tile([128, 4, CH], F8)
                    for p in range(4):  # head pair p -> heads 2p, 2p+1
                        # simT (x1024) for both heads, bf16 matmuls
                        pts = [ps2.tile([128, 2, CH], F32, tag="sim",
                                        name=f"psim{p}_{hh2}")
                               for hh2 in range(2)]
                        for j in range(2):
                            for hh in range(2):
                                h0 = hh * 64
                                nc.tensor.matmul(
                                    pts[hh][:, j, :],
                                    kT_sb[h0:h0 + 64, p, bass.ts(j, 128)],
                                    q_sb[h0:h0 + 64, p, :],
                                    start=True,
                                    stop=True,
                                )
                        # E = exp(simT32 * SCALE/1024) -> bf16, one ACT op/head
                        e_tiles = []
                        for hh in range(2):
                            e_sb = ep.tile([128, 2, CH], BF16, tag="e")
                            nc.scalar.activation(
                                out=e_sb,
                                in_=pts[hh],
                                func=mybir.ActivationFunctionType.Exp,
                                scale=float(SCALE) / 1024.0,
                            )
                            e_tiles.append(e_sb)

                        # attn@v + denominator, col-group pairs (bf16)
                        pav = ps.tile([128, CH], F32, tag="ps")
                        pS = ps.tile([128, CH], F32, tag="ps")
                        for kj in range(2):
                            for hh in range(2):
                                h = 2 * p + hh
                                h0 = hh * 64
                                nc.tensor.matmul(
                                    pav[h0:h0 + 64, :],
                                    v_sb[:, kj, bass.ds(h * 64, 64)],
                                    e_tiles[hh][:, kj, :],
                                    start=(kj == 0),
                                    stop=(kj == 1),
                                    skip_group_check=True,
                                )
                        for kj in range(2):
                            for hh in range(2):
                                h0 = hh * 64
                                nc.tensor.matmul(
                                    pS[h0:h0 + 64, :],
                                    ones_sb,
                                    e_tiles[hh][:, kj, :],
                                    start=(kj == 0),
                                    stop=(kj == 1),
                                    skip_group_check=True,
                                )
                        # normalize: oc = pav / pS  (full 128-width), fp8 out
                        r_sb = rp.tile([128, CH], F32, tag="r")
                        nc.vector.reciprocal_approx_fast(out=r_sb, in_=pS)
                        nc.vector.tensor_mul(out=oc_sb[:, p, :], in0=pav, in1=r_sb)

                    # out projection (x32) + residual 32*(x+bout), bf16 out
                    o_sb = outp.tile([128, 4, CH], BF16)
                    for m in range(4):
                        pt = ps.tile([128, CH], F32, tag="ps")
                        for u in range(2):
                            nc.tensor.matmul(
                                pt,
                                wo_sb[:, 2 * u:2 * u + 2, bass.ts(m, 128)],
                                oc_sb[:, 2 * u:2 * u + 2, :],
                                start=(u == 0),
                                stop=(u == 1),
                                perf_mode=DR,
                            )
                        nc.vector.tensor_add(
                            out=o_sb[:, m, :],
                            in0=pt,
                            in1=xr_sb[:, m, :],
                        )
                    nc.gpsimd.dma_start(out=out_t[:, :, isl], in_=o_sb)

    nc.compile()
    return nc


_NC_CACHE = None


def _get_nc():
    global _NC_CACHE
    if _NC_CACHE is None:
        _NC_CACHE = build_bass()
    return _NC_CACHE


def make_in_maps(x, context, Wq, Wkv, Wout, bout):
    """Host-side prep: shard over batch, pre-transpose weights, quantize fp8."""
    f = np.float32
    bf = ml_dtypes.bfloat16
    f8 = ml_dtypes.float8_e4m3
    wqT = np.ascontiguousarray(Wq.T * np.float32(32)).astype(f8)
    wkT = np.ascontiguousarray(Wkv[:512].T * np.float32(32)).astype(f8)
    wvT = np.ascontiguousarray(Wkv[512:].T * np.float32(32)).astype(f8)
    woT = np.ascontiguousarray(Wout.T * np.float32(32)).astype(f8)
    bout = np.asarray(bout, dtype=f)
    in_maps = []
    for b in range(B):
        xf = np.ascontiguousarray(x[b].reshape(DIM, HW), dtype=f)
        in_maps.append({
            "x8": xf.astype(f8),
            "xres32": ((xf + bout[:, None]) * np.float32(32)).astype(bf),
            "ctxT8": np.ascontiguousarray(context[b].T).astype(f8),
            "wqT8": wqT,
            "wkT8": wkT,
            "wvT8": wvT,
            "woT8": woT,
        })
    return in_maps


def kernel(x, context, Wq, Wkv, Wout, bout):
    x = np.asarray(x)
    context = np.asarray(context)
    nc = _get_nc()
    in_maps = make_in_maps(x, context, np.asarray(Wq), np.asarray(Wkv),
                           np.asarray(Wout), np.asarray(bout))
    res = run_bass_kernel_spmd(nc, in_maps, core_ids=list(range(B)))
    out = np.stack([res.results[b]["out"] for b in range(B)], axis=0)
    return (out.astype(np.float32) / np.float32(32)).reshape(B, DIM, 64, 64)
